# revision 45
# baseline (speedup 1.0000x reference)
"""Canny filter (blur + sobel + NMS + double-threshold hysteresis) on 8 trn2 cores.

Sharding: batch dim (8) across 8 cores; each core processes 3 full 640x640
channel images (channel index == per-core image index). All 3 images are
batched side-by-side in the free dimension of every tile, so each engine
instruction covers 3x640 elements and per-instruction overhead amortizes.

Two device launches:
  K1: 5x5 gaussian blur + sobel via separable stages (vertical parts as
      edge-corrected band-matrix matmuls on PE, horizontal parts as
      shifted-tap DVE/POOL ops), squared-gradient-magnitude NMS with
      direction binning via compares (no atan2), per-image min/max stats.
      Writes nms2 = kept mag^2 (0 elsewhere) and stats.
  host: global per-channel min/max -> low/high thresholds (6 floats).
  K2: double threshold + 3x3 strong-dilation hysteresis (bf16 binary masks).
"""
import sys
import numpy as np

for _p in ("/opt/trn_rl_repo", "/root/.axon_site/_ro/trn_rl_repo"):
    if _p not in sys.path:
        sys.path.append(_p)

import concourse.bacc as bacc
import concourse.tile as tile
from concourse import mybir
from concourse.bass_utils import run_bass_kernel_spmd

F32 = mybir.dt.float32
BF16 = mybir.dt.bfloat16
ALU = mybir.AluOpType
AXX = mybir.AxisListType.X

H = 640
NI = 3            # images per core
NCORES = 8

# ---------------- host-side constants ----------------
_ii = np.arange(5.0) - 2
_g1d = np.exp(-(_ii ** 2) / 2.0)
_g1d /= _g1d.sum()                       # f64 1D gaussian (outer product == ref 2D kernel)
C0 = float(np.float32(_g1d[2] / _g1d[1]))   # horizontal-blur tap ratios (scale 1/g1)
C2 = float(np.float32(_g1d[0] / _g1d[1]))
G1 = float(_g1d[1])                      # scale bookkeeping
T1SQ = float(np.float32(np.tan(np.deg2rad(22.5)) ** 2))
T2SQ = float(np.float32(np.tan(np.deg2rad(67.5)) ** 2))


def _band_matrices():
    """A_x = Vx@G, A_y = Vy@G: 640x640 edge-corrected vertical operators."""
    G = np.zeros((H, H))
    for d in range(-2, 3):
        i = np.arange(max(0, -d), min(H, H - d))
        G[i, i + d] = _g1d[d + 2]
    Vx = np.zeros((H, H))
    Vy = np.zeros((H, H))
    for d, w in ((-1, 1.0), (0, 2.0), (1, 1.0)):
        i = np.arange(max(0, -d), min(H, H - d))
        Vx[i, i + d] = w
    for d, w in ((-1, -1.0), (1, 1.0)):
        i = np.arange(max(0, -d), min(H, H - d))
        Vy[i, i + d] = w
    return (Vx @ G), (Vy @ G)


# K1 strip geometry: (a, K, r0, M) ; NMS rows = [r0+1, r0+M-1)
K1_STRIPS = [(0, 128, 0, 121)] + [(120 * k - 4, 128, 120 * k - 1, 122) for k in range(1, 5)] + [(596, 44, 599, 41)]
# K2 strip geometry: (lo, n_load, p_off, out_lo, n_out)
K2_STRIPS = [(0, 127, 1, 0, 126)] + [(126 * k - 1, 128, 0, 126 * k, 126) for k in range(1, 5)] + [(629, 11, 0, 630, 10)]


def _k1_band_inputs():
    Ax, Ay = _band_matrices()
    out = {}
    for nm, A in (("bx", Ax), ("by", Ay)):
        for tag, (a, K, r0, M) in (("t", K1_STRIPS[0]), ("i", K1_STRIPS[1]), ("b", K1_STRIPS[5])):
            out[nm + tag] = np.ascontiguousarray(A[r0:r0 + M, a:a + K].T.astype(np.float32))
    return out


def build_k1():
    nc = bacc.Bacc()
    x_d = nc.dram_tensor("x", [NI, H, H], F32, kind="ExternalInput")
    bands = {}
    for nm, (a, K, r0, M) in (("bxt", K1_STRIPS[0]), ("bxi", K1_STRIPS[1]), ("bxb", K1_STRIPS[5]),
                              ("byt", K1_STRIPS[0]), ("byi", K1_STRIPS[1]), ("byb", K1_STRIPS[5])):
        bands[nm] = nc.dram_tensor(nm, [K, M], F32, kind="ExternalInput")
    nms_d = nc.dram_tensor("nms2", [NI, H, H], F32, kind="ExternalOutput")
    stats_d = nc.dram_tensor("stats", [NI, 2, 128], F32, kind="ExternalOutput")

    WB = 646   # padded block width in x/W tiles (3 data cols pad each side)
    VB = 644   # padded block width in vaS/vbS (2 pads each side)

    with tile.TileContext(nc) as tc:
        with (
            tc.tile_pool(name="singles", bufs=1) as singles,
            tc.tile_pool(name="xin", bufs=2) as xpool,
            tc.tile_pool(name="early", bufs=2) as pe_,
            tc.tile_pool(name="work", bufs=1) as pw,
            tc.tile_pool(name="statp", bufs=2) as statp,
            tc.tile_pool(name="psum", bufs=2, space="PSUM") as psp,
        ):
            bt = {}
            for nm in bands:
                t = singles.tile(list(bands[nm].shape), F32, tag=nm)
                nc.sync.dma_start(out=t, in_=bands[nm][:, :])
                bt[nm] = t
            zrow = singles.tile([1, 640], F32, tag="zrow")
            nc.vector.memset(zrow[0:1, :], 0.0)

            maxsl = statp.tile([128, NI, 8], F32, tag="maxsl")
            minsl = statp.tile([128, NI, 8], F32, tag="minsl")
            nc.vector.memset(maxsl[:, :, :], 0.0)
            nc.vector.memset(minsl[:, :, :], 3.0e38)

            for k, (a, K, r0, M) in enumerate(K1_STRIPS):
                bx = bt["bxt" if k == 0 else ("bxb" if k == 5 else "bxi")]
                by = bt["byt" if k == 0 else ("byb" if k == 5 else "byi")]

                xt = xpool.tile([128, NI, WB], F32, tag="xt")
                nc.vector.memset(xt[0:K, :, 0:3], 0.0)
                nc.vector.memset(xt[0:K, :, 643:646], 0.0)
                for i in range(NI):
                    eng = (nc.sync, nc.scalar, nc.sync)[i]
                    eng.dma_start(out=xt[0:K, i, 3:643], in_=x_d[i, a:a + K, :])

                # horizontal blur W = (x*c0) + ((s2*c2) + s1)   (scale 1/g1)
                s1 = pe_.tile([128, NI, 640], F32, tag="s1")
                s2 = pe_.tile([128, NI, 640], F32, tag="s2")
                nc.gpsimd.tensor_tensor(out=s1[0:K, :, :], in0=xt[0:K, :, 2:642], in1=xt[0:K, :, 4:644], op=ALU.add)
                nc.gpsimd.tensor_tensor(out=s2[0:K, :, :], in0=xt[0:K, :, 1:641], in1=xt[0:K, :, 5:645], op=ALU.add)
                tb = s2
                nc.vector.scalar_tensor_tensor(out=tb[0:K, :, :], in0=s2[0:K, :, :], scalar=C2, in1=s1[0:K, :, :], op0=ALU.mult, op1=ALU.add)
                Wt = pe_.tile([128, NI, WB], F32, tag="Wt")
                nc.vector.memset(Wt[0:K, :, 0:3], 0.0)
                nc.vector.memset(Wt[0:K, :, 643:646], 0.0)
                nc.vector.scalar_tensor_tensor(out=Wt[0:K, :, 3:643], in0=xt[0:K, :, 3:643], scalar=C0, in1=tb[0:K, :, :], op0=ALU.mult, op1=ALU.add)

                # vertical band conv on PE (fp32), per image so PSUM tiles are
                # small enough (2 banks) to double-buffer: PE on image i+1
                # overlaps the PSUM->SBUF copy of image i.
                vaS = pw.tile([128, NI, VB], F32, tag="vaS")
                vbS = pw.tile([128, NI, VB], F32, tag="vbS")
                for i in range(NI):
                    va = psp.tile([128, WB], F32, tag="va")
                    vb = psp.tile([128, WB], F32, tag="vb")
                    for c0_ in range(0, WB, 512):
                        c1_ = min(c0_ + 512, WB)
                        nc.tensor.matmul(out=va[0:M, c0_:c1_], lhsT=bx[0:K, 0:M], rhs=Wt[0:K, i, c0_:c1_], start=True, stop=True)
                    for c0_ in range(0, WB, 512):
                        c1_ = min(c0_ + 512, WB)
                        nc.tensor.matmul(out=vb[0:M, c0_:c1_], lhsT=by[0:K, 0:M], rhs=Wt[0:K, i, c0_:c1_], start=True, stop=True)
                    nc.scalar.copy(out=vaS[0:M, i, :], in_=va[0:M, 2:646])
                    nc.scalar.copy(out=vbS[0:M, i, :], in_=vb[0:M, 2:646])

                # horizontal sobels (pads give exact zero-pad borders)
                gx = pw.tile([128, NI, 640], F32, tag="gx")
                nc.vector.tensor_tensor(out=gx[0:M, :, :], in0=vaS[0:M, :, 2:642], in1=vaS[0:M, :, 0:640], op=ALU.subtract)
                v1 = pw.tile([128, NI, 640], F32, tag="v1")
                nc.vector.tensor_tensor(out=v1[0:M, :, :], in0=vbS[0:M, :, 0:640], in1=vbS[0:M, :, 2:642], op=ALU.add)
                gy = v1
                nc.vector.scalar_tensor_tensor(out=gy[0:M, :, :], in0=vbS[0:M, :, 1:641], scalar=2.0, in1=v1[0:M, :, :], op0=ALU.mult, op1=ALU.add)

                # squared magnitude + per-image stats
                sx = pw.tile([128, NI, 640], F32, tag="sx")
                sy = pw.tile([128, NI, 640], F32, tag="sy")
                nc.scalar.square(out=sx[0:M, :, :], in_=gx[0:M, :, :])
                nc.scalar.square(out=sy[0:M, :, :], in_=gy[0:M, :, :])
                b0m = pw.tile([128, NI, 638], BF16, tag="b0m")
                b2m = pw.tile([128, NI, 638], BF16, tag="b2m")
                nc.vector.scalar_tensor_tensor(out=b0m[0:M, :, :], in0=sx[0:M, :, 1:639], scalar=T1SQ, in1=sy[0:M, :, 1:639], op0=ALU.mult, op1=ALU.is_gt)
                nc.vector.scalar_tensor_tensor(out=b2m[0:M, :, :], in0=sx[0:M, :, 1:639], scalar=T2SQ, in1=sy[0:M, :, 1:639], op0=ALU.mult, op1=ALU.is_le)
                sxs = pw.tile([128, NI, 638], BF16, tag="sxs")
                sys_ = pw.tile([128, NI, 638], BF16, tag="sys")
                nc.scalar.sign(out=sxs[0:M, :, :], in_=gx[0:M, :, 1:639])
                nc.scalar.sign(out=sys_[0:M, :, :], in_=gy[0:M, :, 1:639])
                sp = pw.tile([128, NI, 638], BF16, tag="sgn")
                nc.vector.tensor_tensor(out=sp[0:M, :, :], in0=sxs[0:M, :, :], in1=sys_[0:M, :, :], op=ALU.is_equal)
                m2 = pe_.tile([128, NI, 640], F32, tag="m2")
                nc.vector.tensor_tensor(out=m2[0:M, :, :], in0=sx[0:M, :, :], in1=sy[0:M, :, :], op=ALU.add)
                nc.vector.tensor_reduce(out=maxsl[0:M, :, k], in_=m2[0:M, :, :], axis=AXX, op=ALU.max)
                nc.vector.tensor_reduce(out=minsl[0:M, :, k], in_=m2[0:M, :, :], axis=AXX, op=ALU.min)

                # vertical-shifted copies via DMA (compute needs partition-base 0)
                dn = pe_.tile([128, NI, 640], F32, tag="dn")
                sh = pe_.tile([128, NI, 640], F32, tag="sh")
                nc.scalar.dma_start(out=dn[0:1, :, :], in_=m2[0:1, :, :])
                nc.scalar.dma_start(out=dn[1:M, :, :], in_=m2[0:M - 1, :, :])
                nc.gpsimd.dma_start(out=sh[M - 1:M, :, :], in_=m2[M - 1:M, :, :])
                nc.gpsimd.dma_start(out=sh[0:M - 1, :, :], in_=m2[1:M, :, :])

                # NMS (valid partitions [1, M-1), image cols [1, 639))
                p0 = pw.tile([128, NI, 638], F32, tag="p0")
                p1 = pw.tile([128, NI, 638], F32, tag="p1")
                p2 = pw.tile([128, NI, 638], F32, tag="p2")
                p3 = pw.tile([128, NI, 638], F32, tag="p3")
                nc.vector.tensor_tensor(out=p0[0:M, :, :], in0=m2[0:M, :, 0:638], in1=m2[0:M, :, 2:640], op=ALU.max)
                nc.vector.tensor_tensor(out=p1[0:M, :, :], in0=sh[0:M, :, 0:638], in1=dn[0:M, :, 2:640], op=ALU.max)
                nc.vector.tensor_tensor(out=p2[0:M, :, :], in0=sh[0:M, :, 1:639], in1=dn[0:M, :, 1:639], op=ALU.max)
                nc.vector.tensor_tensor(out=p3[0:M, :, :], in0=sh[0:M, :, 2:640], in1=dn[0:M, :, 0:638], op=ALU.max)

                pd = p3
                nc.vector.copy_predicated(out=pd[0:M, :, :], mask=sp[0:M, :, :].bitcast(mybir.dt.int16), data=p1[0:M, :, :])
                nc.vector.copy_predicated(out=pd[0:M, :, :], mask=b0m[0:M, :, :].bitcast(mybir.dt.int16), data=p0[0:M, :, :])
                nc.vector.copy_predicated(out=pd[0:M, :, :], mask=b2m[0:M, :, :].bitcast(mybir.dt.int16), data=p2[0:M, :, :])

                km = pw.tile([128, NI, 638], F32, tag="v1")
                nc.vector.tensor_tensor(out=km[0:M, :, :], in0=m2[0:M, :, 1:639], in1=pd[0:M, :, :], op=ALU.is_ge)
                nm = pw.tile([128, NI, 640], F32, tag="gx")
                nc.vector.tensor_tensor(out=nm[0:M, :, 1:639], in0=km[0:M, :, :], in1=m2[0:M, :, 1:639], op=ALU.mult)

                # borders + output
                nc.vector.memset(nm[0:M, :, 0:1], 0.0)
                nc.vector.memset(nm[0:M, :, 639:640], 0.0)
                if k == 0:
                    nc.gpsimd.memset(nm[0:1, :, :], 0.0)
                    plo, phi, rlo = 0, 120, 0
                elif k == 5:
                    plo, phi, rlo = 1, 40, 600
                else:
                    plo, phi, rlo = 1, 121, 120 * k
                for i in range(NI):
                    eng = (nc.sync, nc.scalar, nc.scalar)[i]
                    eng.dma_start(out=nms_d[i, rlo:rlo + (phi - plo), :], in_=nm[plo:phi, i, :])
                if k == 5:
                    for i in range(NI):
                        nc.sync.dma_start(out=nms_d[i, 639:640, :], in_=zrow[0:1, :])

            st = statp.tile([128, NI, 2], F32, tag="st")
            nc.vector.tensor_reduce(out=st[:, :, 0], in_=minsl[:, :, 0:6], axis=AXX, op=ALU.min)
            nc.vector.tensor_reduce(out=st[:, :, 1], in_=maxsl[:, :, 0:6], axis=AXX, op=ALU.max)
            for i in range(NI):
                nc.sync.dma_start(out=stats_d[i].rearrange("s p -> p s"), in_=st[:, i, :])
    nc.compile()
    return nc


def build_k2(tl2, th2):
    """tl2/th2: per-image (== per-channel) squared thresholds, device scale."""
    nc = bacc.Bacc()
    nms_d = nc.dram_tensor("nms2", [NI, H, H], F32, kind="ExternalInput")
    tri_d = nc.dram_tensor("tri", [128, 128], BF16, kind="ExternalInput")
    edges_d = nc.dram_tensor("edges", [NI, H, H], BF16, kind="ExternalOutput")
    th_in = [float(v) for v in th2]
    tl_in = [float(v) for v in tl2]
    assert len(th_in) == NI and len(tl_in) == NI

    with tile.TileContext(nc) as tc:
        with (
            tc.tile_pool(name="k2singles", bufs=1) as k2s,
            tc.tile_pool(name="work", bufs=4) as pw,
            tc.tile_pool(name="psum", bufs=2, space="PSUM") as psp,
        ):
            tri = k2s.tile([128, 128], BF16, tag="tri")
            nc.sync.dma_start(out=tri, in_=tri_d[:, :])
            for k, (lo, nl, poff, olo, nout) in enumerate(K2_STRIPS):
                PT = poff + nl + (1 if k == 5 else 0)   # wt partitions = rows [olo-1, olo+nout+1)
                wt = pw.tile([128, NI, 642], F32, tag="wt")
                nc.vector.memset(wt[0:PT, :, 0:1], 0.0)
                nc.vector.memset(wt[0:PT, :, 641:642], 0.0)
                if k == 0:
                    nc.vector.memset(wt[0:1, :, 1:641], 0.0)
                for i in range(NI):
                    eng = (nc.sync, nc.scalar, nc.gpsimd)[i]
                    if k == 5:
                        eng.dma_start(out=wt[11:12, i, 1:641], in_=nms_d[i, 639:640, :])
                    eng.dma_start(out=wt[poff:poff + nl, i, 1:641], in_=nms_d[i, lo:lo + nl, :])

                strong = pw.tile([128, NI, 642], BF16, tag="strong")
                wlo = pw.tile([128, NI, 642], BF16, tag="wlo")
                for i in range(NI):
                    nc.vector.tensor_scalar(out=strong[0:PT, i, :], in0=wt[0:PT, i, :], scalar1=th_in[i], scalar2=None, op0=ALU.is_ge)
                    nc.vector.tensor_scalar(out=wlo[0:PT, i, :], in0=wt[0:PT, i, :], scalar1=tl_in[i], scalar2=None, op0=ALU.is_ge)

                # horizontal dilation on DVE, vertical via PE tridiagonal band
                # (sum of binary neighbors > 0 == max, nonneg)
                d1 = pw.tile([128, NI, 640], BF16, tag="d1")
                nc.vector.tensor_tensor(out=d1[0:PT, :, :], in0=strong[0:PT, :, 0:640], in1=strong[0:PT, :, 2:642], op=ALU.max)
                h3 = pw.tile([128, NI, 640], BF16, tag="h3")
                nc.vector.tensor_tensor(out=h3[0:PT, :, :], in0=d1[0:PT, :, :], in1=strong[0:PT, :, 1:641], op=ALU.max)
                h3f = h3[0:PT, :, :].rearrange("p i c -> p (i c)")
                NF2 = NI * 640
                vd = psp.tile([128, NF2], F32, tag="vd")
                for c0_ in range(0, NF2, 512):
                    c1_ = min(c0_ + 512, NF2)
                    nc.tensor.matmul(out=vd[0:PT, c0_:c1_], lhsT=tri[0:PT, 0:PT], rhs=h3f[:, c0_:c1_], start=True, stop=True)

                q = pw.tile([128, NI, 640], BF16, tag="q")
                nc.gpsimd.tensor_tensor(out=q[0:PT, :, :], in0=wlo[0:PT, :, 1:641], in1=strong[0:PT, :, 1:641], op=ALU.subtract)
                # vd >= 0 always, so Sign(vd) is exactly the binary dilation mask
                vdv = vd.rearrange("p (i c) -> p i c", i=NI)
                v3m = pw.tile([128, NI, 640], BF16, tag="v3m")
                nc.scalar.sign(out=v3m[0:PT, :, :], in_=vdv[0:PT, :, :])
                t2 = pw.tile([128, NI, 640], BF16, tag="t2")
                nc.vector.tensor_tensor(out=t2[0:PT, :, :], in0=v3m[0:PT, :, :], in1=q[0:PT, :, :], op=ALU.mult)
                ed = pw.tile([128, NI, 640], BF16, tag="ed")
                nc.vector.tensor_tensor(out=ed[0:PT, :, :], in0=strong[0:PT, :, 1:641], in1=t2[0:PT, :, :], op=ALU.add)

                for i in range(NI):
                    eng = (nc.sync, nc.scalar, nc.gpsimd)[i]
                    eng.dma_start(out=edges_d[i, olo:olo + nout, :], in_=ed[1:1 + nout, i, :])
    nc.compile()
    return nc


def kernel(x):
    x = np.asarray(x)
    assert x.shape == (NCORES, NI, H, H), x.shape
    xf = np.ascontiguousarray(x.astype(np.float32, copy=False))
    bands = _k1_band_inputs()

    nc1 = build_k1()
    in_maps1 = []
    for i in range(NCORES):
        m = {"x": np.ascontiguousarray(xf[i])}
        m.update(bands)
        in_maps1.append(m)
    r1 = run_bass_kernel_spmd(nc1, in_maps1, core_ids=list(range(NCORES)))
    nms2 = [np.asarray(r["nms2"]) for r in r1.results]
    stats = np.stack([np.asarray(r["stats"]) for r in r1.results])  # [8, 3, 2, 128]

    mn_dev = stats[:, :, 0, :].min(axis=(0, 2)).astype(np.float64)  # per channel
    mx_dev = stats[:, :, 1, :].max(axis=(0, 2)).astype(np.float64)
    mn = np.sqrt(mn_dev) * G1
    mx = np.sqrt(mx_dev) * G1
    tl = mn + 0.1 * (mx - mn + 1e-8)
    th = mn + 0.3 * (mx - mn + 1e-8)
    tl2 = np.float32((tl / G1) ** 2)
    th2 = np.float32((th / G1) ** 2)

    nc2 = build_k2(tl2, th2)
    tri = np.zeros((128, 128), np.float32)
    for d in (-1, 0, 1):
        i = np.arange(max(0, -d), min(128, 128 - d))
        tri[i + d, i] = 1.0          # lhsT[k, m] = 1 where |k - m| <= 1
    import ml_dtypes
    tri = tri.astype(ml_dtypes.bfloat16)
    in_maps2 = [{"nms2": np.ascontiguousarray(nms2[i]), "tri": tri} for i in range(NCORES)]
    r2 = run_bass_kernel_spmd(nc2, in_maps2, core_ids=list(range(NCORES)))
    edges = np.stack([np.asarray(r["edges"]).astype(np.float32) for r in r2.results])
    return edges


# revision 47
# speedup vs baseline: 1.0308x; 1.0308x over previous
"""Canny filter (blur + sobel + NMS + double-threshold hysteresis) on 8 trn2 cores.

Sharding: batch dim (8) across 8 cores; each core processes 3 full 640x640
channel images (channel index == per-core image index). All 3 images are
batched side-by-side in the free dimension of every tile, so each engine
instruction covers 3x640 elements and per-instruction overhead amortizes.

Two device launches:
  K1: 5x5 gaussian blur + sobel via separable stages (vertical parts as
      edge-corrected band-matrix matmuls on PE, horizontal parts as
      shifted-tap DVE/POOL ops), squared-gradient-magnitude NMS with
      direction binning via compares (no atan2), per-image min/max stats.
      Writes nms2 = kept mag^2 (0 elsewhere) and stats.
  host: global per-channel min/max -> low/high thresholds (6 floats).
  K2: double threshold + 3x3 strong-dilation hysteresis (bf16 binary masks).
"""
import sys
import numpy as np

for _p in ("/opt/trn_rl_repo", "/root/.axon_site/_ro/trn_rl_repo"):
    if _p not in sys.path:
        sys.path.append(_p)

import concourse.bacc as bacc
import concourse.tile as tile
from concourse import mybir
from concourse.bass_utils import run_bass_kernel_spmd

F32 = mybir.dt.float32
BF16 = mybir.dt.bfloat16
ALU = mybir.AluOpType
AXX = mybir.AxisListType.X

H = 640
NI = 3            # images per core
NCORES = 8

# ---------------- host-side constants ----------------
_ii = np.arange(5.0) - 2
_g1d = np.exp(-(_ii ** 2) / 2.0)
_g1d /= _g1d.sum()                       # f64 1D gaussian (outer product == ref 2D kernel)
C0 = float(np.float32(_g1d[2] / _g1d[1]))   # horizontal-blur tap ratios (scale 1/g1)
C2 = float(np.float32(_g1d[0] / _g1d[1]))
G1 = float(_g1d[1])                      # scale bookkeeping
T1SQ = float(np.float32(np.tan(np.deg2rad(22.5)) ** 2))
T2SQ = float(np.float32(np.tan(np.deg2rad(67.5)) ** 2))


def _band_matrices():
    """A_x = Vx@G, A_y = Vy@G: 640x640 edge-corrected vertical operators."""
    G = np.zeros((H, H))
    for d in range(-2, 3):
        i = np.arange(max(0, -d), min(H, H - d))
        G[i, i + d] = _g1d[d + 2]
    Vx = np.zeros((H, H))
    Vy = np.zeros((H, H))
    for d, w in ((-1, 1.0), (0, 2.0), (1, 1.0)):
        i = np.arange(max(0, -d), min(H, H - d))
        Vx[i, i + d] = w
    for d, w in ((-1, -1.0), (1, 1.0)):
        i = np.arange(max(0, -d), min(H, H - d))
        Vy[i, i + d] = w
    return (Vx @ G), (Vy @ G)


# K1 strip geometry: (a, K, r0, M) ; NMS rows = [r0+1, r0+M-1)
K1_STRIPS = [(0, 128, 0, 121)] + [(120 * k - 4, 128, 120 * k - 1, 122) for k in range(1, 5)] + [(596, 44, 599, 41)]
# K2 strip geometry: (lo, n_load, p_off, out_lo, n_out)
K2_STRIPS = [(0, 127, 1, 0, 126)] + [(126 * k - 1, 128, 0, 126 * k, 126) for k in range(1, 5)] + [(629, 11, 0, 630, 10)]


def _k1_band_inputs():
    Ax, Ay = _band_matrices()
    out = {}
    for nm, A in (("bx", Ax), ("by", Ay)):
        for tag, (a, K, r0, M) in (("t", K1_STRIPS[0]), ("i", K1_STRIPS[1]), ("b", K1_STRIPS[5])):
            out[nm + tag] = np.ascontiguousarray(A[r0:r0 + M, a:a + K].T.astype(np.float32))
    return out


def build_k1():
    nc = bacc.Bacc()
    x_d = nc.dram_tensor("x", [NI, H, H], F32, kind="ExternalInput")
    bands = {}
    for nm, (a, K, r0, M) in (("bxt", K1_STRIPS[0]), ("bxi", K1_STRIPS[1]), ("bxb", K1_STRIPS[5]),
                              ("byt", K1_STRIPS[0]), ("byi", K1_STRIPS[1]), ("byb", K1_STRIPS[5])):
        bands[nm] = nc.dram_tensor(nm, [K, M], F32, kind="ExternalInput")
    nms_d = nc.dram_tensor("nms2", [NI, H, H], F32, kind="ExternalOutput")
    stats_d = nc.dram_tensor("stats", [NI, 2, 128], F32, kind="ExternalOutput")

    WB = 646   # padded block width in x/W tiles (3 data cols pad each side)
    VB = 644   # padded block width in vaS/vbS (2 pads each side)

    with tile.TileContext(nc) as tc:
        with (
            tc.tile_pool(name="singles", bufs=1) as singles,
            tc.tile_pool(name="xin", bufs=3) as xpool,
            tc.tile_pool(name="early", bufs=2) as pe_,
            tc.tile_pool(name="work", bufs=1) as pw,
            tc.tile_pool(name="statp", bufs=2) as statp,
            tc.tile_pool(name="psum", bufs=2, space="PSUM") as psp,
        ):
            bt = {}
            for nm in bands:
                t = singles.tile(list(bands[nm].shape), F32, tag=nm)
                nc.sync.dma_start(out=t, in_=bands[nm][:, :])
                bt[nm] = t
            zrow = singles.tile([1, 640], F32, tag="zrow")
            nc.vector.memset(zrow[0:1, :], 0.0)

            maxsl = statp.tile([128, NI, 8], F32, tag="maxsl")
            minsl = statp.tile([128, NI, 8], F32, tag="minsl")
            nc.vector.memset(maxsl[:, :, :], 0.0)
            nc.vector.memset(minsl[:, :, :], 3.0e38)

            for k, (a, K, r0, M) in enumerate(K1_STRIPS):
                bx = bt["bxt" if k == 0 else ("bxb" if k == 5 else "bxi")]
                by = bt["byt" if k == 0 else ("byb" if k == 5 else "byi")]

                xt = xpool.tile([128, NI, WB], F32, tag="xt")
                nc.vector.memset(xt[0:K, :, 0:3], 0.0)
                nc.vector.memset(xt[0:K, :, 643:646], 0.0)
                for i in range(NI):
                    eng = (nc.sync, nc.scalar, nc.sync)[i]
                    eng.dma_start(out=xt[0:K, i, 3:643], in_=x_d[i, a:a + K, :])

                # horizontal blur W = (x*c0) + ((s2*c2) + s1)   (scale 1/g1)
                s1 = pe_.tile([128, NI, 640], F32, tag="s1")
                s2 = pe_.tile([128, NI, 640], F32, tag="s2")
                nc.gpsimd.tensor_tensor(out=s1[0:K, :, :], in0=xt[0:K, :, 2:642], in1=xt[0:K, :, 4:644], op=ALU.add)
                nc.gpsimd.tensor_tensor(out=s2[0:K, :, :], in0=xt[0:K, :, 1:641], in1=xt[0:K, :, 5:645], op=ALU.add)
                tb = s2
                nc.vector.scalar_tensor_tensor(out=tb[0:K, :, :], in0=s2[0:K, :, :], scalar=C2, in1=s1[0:K, :, :], op0=ALU.mult, op1=ALU.add)
                Wt = pe_.tile([128, NI, WB], F32, tag="Wt")
                nc.vector.memset(Wt[0:K, :, 0:3], 0.0)
                nc.vector.memset(Wt[0:K, :, 643:646], 0.0)
                nc.vector.scalar_tensor_tensor(out=Wt[0:K, :, 3:643], in0=xt[0:K, :, 3:643], scalar=C0, in1=tb[0:K, :, :], op0=ALU.mult, op1=ALU.add)

                # vertical band conv on PE (fp32), per image so PSUM tiles are
                # small enough (2 banks) to double-buffer: PE on image i+1
                # overlaps the PSUM->SBUF copy of image i.
                vaS = pw.tile([128, NI, VB], F32, tag="vaS")
                vbS = pw.tile([128, NI, VB], F32, tag="vbS")
                for i in range(NI):
                    va = psp.tile([128, WB], F32, tag="va")
                    vb = psp.tile([128, WB], F32, tag="vb")
                    for c0_ in range(0, WB, 512):
                        c1_ = min(c0_ + 512, WB)
                        nc.tensor.matmul(out=va[0:M, c0_:c1_], lhsT=bx[0:K, 0:M], rhs=Wt[0:K, i, c0_:c1_], start=True, stop=True)
                    for c0_ in range(0, WB, 512):
                        c1_ = min(c0_ + 512, WB)
                        nc.tensor.matmul(out=vb[0:M, c0_:c1_], lhsT=by[0:K, 0:M], rhs=Wt[0:K, i, c0_:c1_], start=True, stop=True)
                    nc.scalar.copy(out=vaS[0:M, i, :], in_=va[0:M, 2:646])
                    nc.scalar.copy(out=vbS[0:M, i, :], in_=vb[0:M, 2:646])

                # horizontal sobels (pads give exact zero-pad borders)
                gx = pw.tile([128, NI, 640], F32, tag="gx")
                nc.vector.tensor_tensor(out=gx[0:M, :, :], in0=vaS[0:M, :, 2:642], in1=vaS[0:M, :, 0:640], op=ALU.subtract)
                v1 = pw.tile([128, NI, 640], F32, tag="v1")
                nc.vector.tensor_tensor(out=v1[0:M, :, :], in0=vbS[0:M, :, 0:640], in1=vbS[0:M, :, 2:642], op=ALU.add)
                gy = v1
                nc.vector.scalar_tensor_tensor(out=gy[0:M, :, :], in0=vbS[0:M, :, 1:641], scalar=2.0, in1=v1[0:M, :, :], op0=ALU.mult, op1=ALU.add)

                # squared magnitude + per-image stats
                sx = pw.tile([128, NI, 640], F32, tag="sx")
                sy = pe_.tile([128, NI, 640], F32, tag="s1")
                nc.scalar.square(out=sx[0:M, :, :], in_=gx[0:M, :, :])
                nc.scalar.square(out=sy[0:M, :, :], in_=gy[0:M, :, :])
                b0m = pw.tile([128, NI, 638], BF16, tag="b0m")
                b2m = pw.tile([128, NI, 638], BF16, tag="b2m")
                nc.vector.scalar_tensor_tensor(out=b0m[0:M, :, :], in0=sx[0:M, :, 1:639], scalar=T1SQ, in1=sy[0:M, :, 1:639], op0=ALU.mult, op1=ALU.is_gt)
                nc.vector.scalar_tensor_tensor(out=b2m[0:M, :, :], in0=sx[0:M, :, 1:639], scalar=T2SQ, in1=sy[0:M, :, 1:639], op0=ALU.mult, op1=ALU.is_le)
                sxs = pw.tile([128, NI, 638], BF16, tag="sxs")
                sys_ = pw.tile([128, NI, 638], BF16, tag="sys")
                nc.scalar.sign(out=sxs[0:M, :, :], in_=gx[0:M, :, 1:639])
                nc.scalar.sign(out=sys_[0:M, :, :], in_=gy[0:M, :, 1:639])
                sp = pw.tile([128, NI, 638], BF16, tag="sgn")
                nc.vector.tensor_tensor(out=sp[0:M, :, :], in0=sxs[0:M, :, :], in1=sys_[0:M, :, :], op=ALU.is_equal)
                m2 = pe_.tile([128, NI, 640], F32, tag="m2")
                nc.vector.tensor_tensor(out=m2[0:M, :, :], in0=sx[0:M, :, :], in1=sy[0:M, :, :], op=ALU.add)
                nc.vector.tensor_reduce(out=maxsl[0:M, :, k], in_=m2[0:M, :, :], axis=AXX, op=ALU.max)
                nc.vector.tensor_reduce(out=minsl[0:M, :, k], in_=m2[0:M, :, :], axis=AXX, op=ALU.min)

                # vertical-shifted copies via DMA (compute needs partition-base 0)
                dn = pe_.tile([128, NI, 640], F32, tag="dn")
                sh = pe_.tile([128, NI, 640], F32, tag="sh")
                nc.scalar.dma_start(out=dn[0:1, :, :], in_=m2[0:1, :, :])
                nc.scalar.dma_start(out=dn[1:M, :, :], in_=m2[0:M - 1, :, :])
                nc.gpsimd.dma_start(out=sh[M - 1:M, :, :], in_=m2[M - 1:M, :, :])
                nc.gpsimd.dma_start(out=sh[0:M - 1, :, :], in_=m2[1:M, :, :])

                # NMS (valid partitions [1, M-1), image cols [1, 639))
                p0 = pw.tile([128, NI, 638], F32, tag="p0")
                p1 = pw.tile([128, NI, 638], F32, tag="p1")
                p2 = pw.tile([128, NI, 638], F32, tag="p2")
                p3 = pw.tile([128, NI, 638], F32, tag="p3")
                nc.vector.tensor_tensor(out=p0[0:M, :, :], in0=m2[0:M, :, 0:638], in1=m2[0:M, :, 2:640], op=ALU.max)
                nc.vector.tensor_tensor(out=p1[0:M, :, :], in0=sh[0:M, :, 0:638], in1=dn[0:M, :, 2:640], op=ALU.max)
                nc.vector.tensor_tensor(out=p2[0:M, :, :], in0=sh[0:M, :, 1:639], in1=dn[0:M, :, 1:639], op=ALU.max)
                nc.vector.tensor_tensor(out=p3[0:M, :, :], in0=sh[0:M, :, 2:640], in1=dn[0:M, :, 0:638], op=ALU.max)

                pd = p3
                nc.vector.copy_predicated(out=pd[0:M, :, :], mask=sp[0:M, :, :].bitcast(mybir.dt.int16), data=p1[0:M, :, :])
                nc.vector.copy_predicated(out=pd[0:M, :, :], mask=b0m[0:M, :, :].bitcast(mybir.dt.int16), data=p0[0:M, :, :])
                nc.vector.copy_predicated(out=pd[0:M, :, :], mask=b2m[0:M, :, :].bitcast(mybir.dt.int16), data=p2[0:M, :, :])

                km = pw.tile([128, NI, 638], F32, tag="v1")
                nc.vector.tensor_tensor(out=km[0:M, :, :], in0=m2[0:M, :, 1:639], in1=pd[0:M, :, :], op=ALU.is_ge)
                nm = pw.tile([128, NI, 640], F32, tag="gx")
                nc.vector.tensor_tensor(out=nm[0:M, :, 1:639], in0=km[0:M, :, :], in1=m2[0:M, :, 1:639], op=ALU.mult)

                # borders + output
                nc.vector.memset(nm[0:M, :, 0:1], 0.0)
                nc.vector.memset(nm[0:M, :, 639:640], 0.0)
                if k == 0:
                    nc.gpsimd.memset(nm[0:1, :, :], 0.0)
                    plo, phi, rlo = 0, 120, 0
                elif k == 5:
                    plo, phi, rlo = 1, 40, 600
                else:
                    plo, phi, rlo = 1, 121, 120 * k
                for i in range(NI):
                    eng = (nc.sync, nc.scalar, nc.scalar)[i]
                    eng.dma_start(out=nms_d[i, rlo:rlo + (phi - plo), :], in_=nm[plo:phi, i, :])
                if k == 5:
                    for i in range(NI):
                        nc.sync.dma_start(out=nms_d[i, 639:640, :], in_=zrow[0:1, :])

            st = statp.tile([128, NI, 2], F32, tag="st")
            nc.vector.tensor_reduce(out=st[:, :, 0], in_=minsl[:, :, 0:6], axis=AXX, op=ALU.min)
            nc.vector.tensor_reduce(out=st[:, :, 1], in_=maxsl[:, :, 0:6], axis=AXX, op=ALU.max)
            for i in range(NI):
                nc.sync.dma_start(out=stats_d[i].rearrange("s p -> p s"), in_=st[:, i, :])
    nc.compile()
    return nc


def build_k2(tl2, th2):
    """tl2/th2: per-image (== per-channel) squared thresholds, device scale."""
    nc = bacc.Bacc()
    nms_d = nc.dram_tensor("nms2", [NI, H, H], F32, kind="ExternalInput")
    tri_d = nc.dram_tensor("tri", [128, 128], BF16, kind="ExternalInput")
    edges_d = nc.dram_tensor("edges", [NI, H, H], BF16, kind="ExternalOutput")
    th_in = [float(v) for v in th2]
    tl_in = [float(v) for v in tl2]
    assert len(th_in) == NI and len(tl_in) == NI

    with tile.TileContext(nc) as tc:
        with (
            tc.tile_pool(name="k2singles", bufs=1) as k2s,
            tc.tile_pool(name="work", bufs=4) as pw,
            tc.tile_pool(name="psum", bufs=2, space="PSUM") as psp,
        ):
            tri = k2s.tile([128, 128], BF16, tag="tri")
            nc.sync.dma_start(out=tri, in_=tri_d[:, :])
            for k, (lo, nl, poff, olo, nout) in enumerate(K2_STRIPS):
                PT = poff + nl + (1 if k == 5 else 0)   # wt partitions = rows [olo-1, olo+nout+1)
                wt = pw.tile([128, NI, 642], F32, tag="wt")
                nc.vector.memset(wt[0:PT, :, 0:1], 0.0)
                nc.vector.memset(wt[0:PT, :, 641:642], 0.0)
                if k == 0:
                    nc.vector.memset(wt[0:1, :, 1:641], 0.0)
                for i in range(NI):
                    eng = (nc.sync, nc.scalar, nc.gpsimd)[i]
                    if k == 5:
                        eng.dma_start(out=wt[11:12, i, 1:641], in_=nms_d[i, 639:640, :])
                    eng.dma_start(out=wt[poff:poff + nl, i, 1:641], in_=nms_d[i, lo:lo + nl, :])

                strong = pw.tile([128, NI, 642], BF16, tag="strong")
                wlo = pw.tile([128, NI, 642], BF16, tag="wlo")
                for i in range(NI):
                    nc.vector.tensor_scalar(out=strong[0:PT, i, :], in0=wt[0:PT, i, :], scalar1=th_in[i], scalar2=None, op0=ALU.is_ge)
                    nc.vector.tensor_scalar(out=wlo[0:PT, i, :], in0=wt[0:PT, i, :], scalar1=tl_in[i], scalar2=None, op0=ALU.is_ge)

                # horizontal dilation on DVE, vertical via PE tridiagonal band
                # (sum of binary neighbors > 0 == max, nonneg)
                d1 = pw.tile([128, NI, 640], BF16, tag="d1")
                nc.vector.tensor_tensor(out=d1[0:PT, :, :], in0=strong[0:PT, :, 0:640], in1=strong[0:PT, :, 2:642], op=ALU.max)
                h3 = pw.tile([128, NI, 640], BF16, tag="h3")
                nc.vector.tensor_tensor(out=h3[0:PT, :, :], in0=d1[0:PT, :, :], in1=strong[0:PT, :, 1:641], op=ALU.max)
                h3f = h3[0:PT, :, :].rearrange("p i c -> p (i c)")
                NF2 = NI * 640
                vd = psp.tile([128, NF2], F32, tag="vd")
                for c0_ in range(0, NF2, 512):
                    c1_ = min(c0_ + 512, NF2)
                    nc.tensor.matmul(out=vd[0:PT, c0_:c1_], lhsT=tri[0:PT, 0:PT], rhs=h3f[:, c0_:c1_], start=True, stop=True)

                q = pw.tile([128, NI, 640], BF16, tag="q")
                nc.gpsimd.tensor_tensor(out=q[0:PT, :, :], in0=wlo[0:PT, :, 1:641], in1=strong[0:PT, :, 1:641], op=ALU.subtract)
                # vd >= 0 always, so Sign(vd) is exactly the binary dilation mask
                vdv = vd.rearrange("p (i c) -> p i c", i=NI)
                v3m = pw.tile([128, NI, 640], BF16, tag="v3m")
                nc.scalar.sign(out=v3m[0:PT, :, :], in_=vdv[0:PT, :, :])
                t2 = pw.tile([128, NI, 640], BF16, tag="t2")
                nc.vector.tensor_tensor(out=t2[0:PT, :, :], in0=v3m[0:PT, :, :], in1=q[0:PT, :, :], op=ALU.mult)
                ed = pw.tile([128, NI, 640], BF16, tag="ed")
                nc.vector.tensor_tensor(out=ed[0:PT, :, :], in0=strong[0:PT, :, 1:641], in1=t2[0:PT, :, :], op=ALU.add)

                for i in range(NI):
                    eng = (nc.sync, nc.scalar, nc.gpsimd)[i]
                    eng.dma_start(out=edges_d[i, olo:olo + nout, :], in_=ed[1:1 + nout, i, :])
    nc.compile()
    return nc


def kernel(x):
    x = np.asarray(x)
    assert x.shape == (NCORES, NI, H, H), x.shape
    xf = np.ascontiguousarray(x.astype(np.float32, copy=False))
    bands = _k1_band_inputs()

    nc1 = build_k1()
    in_maps1 = []
    for i in range(NCORES):
        m = {"x": np.ascontiguousarray(xf[i])}
        m.update(bands)
        in_maps1.append(m)
    r1 = run_bass_kernel_spmd(nc1, in_maps1, core_ids=list(range(NCORES)))
    nms2 = [np.asarray(r["nms2"]) for r in r1.results]
    stats = np.stack([np.asarray(r["stats"]) for r in r1.results])  # [8, 3, 2, 128]

    mn_dev = stats[:, :, 0, :].min(axis=(0, 2)).astype(np.float64)  # per channel
    mx_dev = stats[:, :, 1, :].max(axis=(0, 2)).astype(np.float64)
    mn = np.sqrt(mn_dev) * G1
    mx = np.sqrt(mx_dev) * G1
    tl = mn + 0.1 * (mx - mn + 1e-8)
    th = mn + 0.3 * (mx - mn + 1e-8)
    tl2 = np.float32((tl / G1) ** 2)
    th2 = np.float32((th / G1) ** 2)

    nc2 = build_k2(tl2, th2)
    tri = np.zeros((128, 128), np.float32)
    for d in (-1, 0, 1):
        i = np.arange(max(0, -d), min(128, 128 - d))
        tri[i + d, i] = 1.0          # lhsT[k, m] = 1 where |k - m| <= 1
    import ml_dtypes
    tri = tri.astype(ml_dtypes.bfloat16)
    in_maps2 = [{"nms2": np.ascontiguousarray(nms2[i]), "tri": tri} for i in range(NCORES)]
    r2 = run_bass_kernel_spmd(nc2, in_maps2, core_ids=list(range(NCORES)))
    edges = np.stack([np.asarray(r["edges"]).astype(np.float32) for r in r2.results])
    return edges


# revision 48
# speedup vs baseline: 1.0694x; 1.0375x over previous
"""Canny filter (blur + sobel + NMS + double-threshold hysteresis) on 8 trn2 cores.

Sharding: batch dim (8) across 8 cores; each core processes 3 full 640x640
channel images (channel index == per-core image index). All 3 images are
batched side-by-side in the free dimension of every tile, so each engine
instruction covers 3x640 elements and per-instruction overhead amortizes.

Two device launches:
  K1: 5x5 gaussian blur + sobel via separable stages (vertical parts as
      edge-corrected band-matrix matmuls on PE, horizontal parts as
      shifted-tap DVE/POOL ops), squared-gradient-magnitude NMS with
      direction binning via compares (no atan2), per-image min/max stats.
      Writes nms2 = kept mag^2 (0 elsewhere) and stats.
  host: global per-channel min/max -> low/high thresholds (6 floats).
  K2: double threshold + 3x3 strong-dilation hysteresis (bf16 binary masks).
"""
import sys
import numpy as np

for _p in ("/opt/trn_rl_repo", "/root/.axon_site/_ro/trn_rl_repo"):
    if _p not in sys.path:
        sys.path.append(_p)

import concourse.bacc as bacc
import concourse.tile as tile
from concourse import mybir
from concourse.bass_utils import run_bass_kernel_spmd

F32 = mybir.dt.float32
BF16 = mybir.dt.bfloat16
ALU = mybir.AluOpType
AXX = mybir.AxisListType.X

H = 640
NI = 3            # images per core
NCORES = 8

# ---------------- host-side constants ----------------
_ii = np.arange(5.0) - 2
_g1d = np.exp(-(_ii ** 2) / 2.0)
_g1d /= _g1d.sum()                       # f64 1D gaussian (outer product == ref 2D kernel)
C0 = float(np.float32(_g1d[2] / _g1d[1]))   # horizontal-blur tap ratios (scale 1/g1)
C2 = float(np.float32(_g1d[0] / _g1d[1]))
G1 = float(_g1d[1])                      # scale bookkeeping
T1SQ = float(np.float32(np.tan(np.deg2rad(22.5)) ** 2))
T2SQ = float(np.float32(np.tan(np.deg2rad(67.5)) ** 2))


def _band_matrices():
    """A_x = Vx@G, A_y = Vy@G: 640x640 edge-corrected vertical operators."""
    G = np.zeros((H, H))
    for d in range(-2, 3):
        i = np.arange(max(0, -d), min(H, H - d))
        G[i, i + d] = _g1d[d + 2]
    Vx = np.zeros((H, H))
    Vy = np.zeros((H, H))
    for d, w in ((-1, 1.0), (0, 2.0), (1, 1.0)):
        i = np.arange(max(0, -d), min(H, H - d))
        Vx[i, i + d] = w
    for d, w in ((-1, -1.0), (1, 1.0)):
        i = np.arange(max(0, -d), min(H, H - d))
        Vy[i, i + d] = w
    return (Vx @ G), (Vy @ G)


# K1 strip geometry: (a, K, r0, M) ; NMS rows = [r0+1, r0+M-1)
K1_STRIPS = [(0, 128, 0, 121)] + [(120 * k - 4, 128, 120 * k - 1, 122) for k in range(1, 5)] + [(596, 44, 599, 41)]
# K2 strip geometry: (lo, n_load, p_off, out_lo, n_out)
K2_STRIPS = [(0, 127, 1, 0, 126)] + [(126 * k - 1, 128, 0, 126 * k, 126) for k in range(1, 5)] + [(629, 11, 0, 630, 10)]


def _k1_band_inputs():
    Ax, Ay = _band_matrices()
    out = {}
    for nm, A in (("bx", Ax), ("by", Ay)):
        for tag, (a, K, r0, M) in (("t", K1_STRIPS[0]), ("i", K1_STRIPS[1]), ("b", K1_STRIPS[5])):
            out[nm + tag] = np.ascontiguousarray(A[r0:r0 + M, a:a + K].T.astype(np.float32))
    return out


def build_k1():
    nc = bacc.Bacc()
    x_d = nc.dram_tensor("x", [NI, H, H], F32, kind="ExternalInput")
    bands = {}
    for nm, (a, K, r0, M) in (("bxt", K1_STRIPS[0]), ("bxi", K1_STRIPS[1]), ("bxb", K1_STRIPS[5]),
                              ("byt", K1_STRIPS[0]), ("byi", K1_STRIPS[1]), ("byb", K1_STRIPS[5])):
        bands[nm] = nc.dram_tensor(nm, [K, M], F32, kind="ExternalInput")
    nms_d = nc.dram_tensor("nms2", [NI, H, H], F32, kind="ExternalOutput")
    stats_d = nc.dram_tensor("stats", [NI, 2, 128], F32, kind="ExternalOutput")

    WB = 646   # padded block width in x/W tiles (3 data cols pad each side)
    VB = 644   # padded block width in vaS/vbS (2 pads each side)

    with tile.TileContext(nc) as tc:
        with (
            tc.tile_pool(name="singles", bufs=1) as singles,
            tc.tile_pool(name="xin", bufs=3) as xpool,
            tc.tile_pool(name="early", bufs=2) as pe_,
            tc.tile_pool(name="work", bufs=1) as pw,
            tc.tile_pool(name="statp", bufs=2) as statp,
            tc.tile_pool(name="psum", bufs=2, space="PSUM") as psp,
        ):
            bt = {}
            for nm in bands:
                t = singles.tile(list(bands[nm].shape), F32, tag=nm)
                nc.sync.dma_start(out=t, in_=bands[nm][:, :])
                bt[nm] = t
            zrow = singles.tile([1, 640], F32, tag="zrow")
            nc.vector.memset(zrow[0:1, :], 0.0)

            maxsl = statp.tile([128, NI, 8], F32, tag="maxsl")
            minsl = statp.tile([128, NI, 8], F32, tag="minsl")
            nc.vector.memset(maxsl[:, :, :], 0.0)
            nc.vector.memset(minsl[:, :, :], 3.0e38)

            for k, (a, K, r0, M) in enumerate(K1_STRIPS):
                bx = bt["bxt" if k == 0 else ("bxb" if k == 5 else "bxi")]
                by = bt["byt" if k == 0 else ("byb" if k == 5 else "byi")]

                xt = xpool.tile([128, NI, WB], F32, tag="xt")
                nc.vector.memset(xt[0:K, :, 0:3], 0.0)
                nc.vector.memset(xt[0:K, :, 643:646], 0.0)
                for i in range(NI):
                    eng = (nc.sync, nc.scalar, nc.sync)[i]
                    eng.dma_start(out=xt[0:K, i, 3:643], in_=x_d[i, a:a + K, :])

                # horizontal blur W = (x*c0) + ((s2*c2) + s1)   (scale 1/g1)
                s1 = pe_.tile([128, NI, 640], F32, tag="s1")
                s2 = pe_.tile([128, NI, 640], F32, tag="s2")
                nc.gpsimd.tensor_tensor(out=s1[0:K, :, :], in0=xt[0:K, :, 2:642], in1=xt[0:K, :, 4:644], op=ALU.add)
                nc.gpsimd.tensor_tensor(out=s2[0:K, :, :], in0=xt[0:K, :, 1:641], in1=xt[0:K, :, 5:645], op=ALU.add)
                tb = s2
                nc.vector.scalar_tensor_tensor(out=tb[0:K, :, :], in0=s2[0:K, :, :], scalar=C2, in1=s1[0:K, :, :], op0=ALU.mult, op1=ALU.add)
                Wt = pe_.tile([128, NI, WB], F32, tag="Wt")
                nc.vector.memset(Wt[0:K, :, 0:3], 0.0)
                nc.vector.memset(Wt[0:K, :, 643:646], 0.0)
                nc.vector.scalar_tensor_tensor(out=Wt[0:K, :, 3:643], in0=xt[0:K, :, 3:643], scalar=C0, in1=tb[0:K, :, :], op0=ALU.mult, op1=ALU.add)

                # vertical band conv on PE (fp32), per image so PSUM tiles are
                # small enough (2 banks) to double-buffer: PE on image i+1
                # overlaps the PSUM->SBUF copy of image i.
                vaS = pw.tile([128, NI, VB], F32, tag="vaS")
                vbS = pw.tile([128, NI, VB], F32, tag="vbS")
                for i in range(NI):
                    va = psp.tile([128, WB], F32, tag="va")
                    vb = psp.tile([128, WB], F32, tag="vb")
                    for c0_ in range(0, WB, 512):
                        c1_ = min(c0_ + 512, WB)
                        nc.tensor.matmul(out=va[0:M, c0_:c1_], lhsT=bx[0:K, 0:M], rhs=Wt[0:K, i, c0_:c1_], start=True, stop=True)
                    for c0_ in range(0, WB, 512):
                        c1_ = min(c0_ + 512, WB)
                        nc.tensor.matmul(out=vb[0:M, c0_:c1_], lhsT=by[0:K, 0:M], rhs=Wt[0:K, i, c0_:c1_], start=True, stop=True)
                    nc.scalar.copy(out=vaS[0:M, i, :], in_=va[0:M, 2:646])
                    nc.scalar.copy(out=vbS[0:M, i, :], in_=vb[0:M, 2:646])

                # horizontal sobels (pads give exact zero-pad borders)
                gx = pw.tile([128, NI, 640], F32, tag="gx")
                nc.vector.tensor_tensor(out=gx[0:M, :, :], in0=vaS[0:M, :, 2:642], in1=vaS[0:M, :, 0:640], op=ALU.subtract)
                v1 = pw.tile([128, NI, 640], F32, tag="v1")
                nc.vector.tensor_tensor(out=v1[0:M, :, :], in0=vbS[0:M, :, 0:640], in1=vbS[0:M, :, 2:642], op=ALU.add)
                gy = v1
                nc.vector.scalar_tensor_tensor(out=gy[0:M, :, :], in0=vbS[0:M, :, 1:641], scalar=2.0, in1=v1[0:M, :, :], op0=ALU.mult, op1=ALU.add)

                # squared magnitude + per-image stats
                sx = pw.tile([128, NI, 640], F32, tag="sx")
                sy = pe_.tile([128, NI, 640], F32, tag="s1")
                nc.scalar.square(out=sx[0:M, :, :], in_=gx[0:M, :, :])
                nc.scalar.square(out=sy[0:M, :, :], in_=gy[0:M, :, :])
                b0m = pw.tile([128, NI, 638], BF16, tag="b0m")
                b2m = pw.tile([128, NI, 638], BF16, tag="b2m")
                nc.vector.scalar_tensor_tensor(out=b0m[0:M, :, :], in0=sx[0:M, :, 1:639], scalar=T1SQ, in1=sy[0:M, :, 1:639], op0=ALU.mult, op1=ALU.is_gt)
                nc.vector.scalar_tensor_tensor(out=b2m[0:M, :, :], in0=sx[0:M, :, 1:639], scalar=T2SQ, in1=sy[0:M, :, 1:639], op0=ALU.mult, op1=ALU.is_le)
                sxs = pw.tile([128, NI, 638], BF16, tag="sxs")
                sys_ = pw.tile([128, NI, 638], BF16, tag="sys")
                nc.scalar.sign(out=sxs[0:M, :, :], in_=gx[0:M, :, 1:639])
                nc.scalar.sign(out=sys_[0:M, :, :], in_=gy[0:M, :, 1:639])
                sp = pw.tile([128, NI, 638], BF16, tag="sgn")
                nc.vector.tensor_tensor(out=sp[0:M, :, :], in0=sxs[0:M, :, :], in1=sys_[0:M, :, :], op=ALU.is_equal)
                m2 = pe_.tile([128, NI, 640], F32, tag="m2")
                nc.vector.tensor_tensor(out=m2[0:M, :, :], in0=sx[0:M, :, :], in1=sy[0:M, :, :], op=ALU.add)
                nc.vector.tensor_reduce(out=maxsl[0:M, :, k], in_=m2[0:M, :, :], axis=AXX, op=ALU.max)
                nc.vector.tensor_reduce(out=minsl[0:M, :, k], in_=m2[0:M, :, :], axis=AXX, op=ALU.min)

                # vertical-shifted copies via DMA (compute needs partition-base 0)
                dn = pe_.tile([128, NI, 640], F32, tag="dn")
                sh = pe_.tile([128, NI, 640], F32, tag="sh")
                nc.scalar.dma_start(out=dn[0:1, :, :], in_=m2[0:1, :, :])
                nc.scalar.dma_start(out=dn[1:M, 0:1, :], in_=m2[0:M - 1, 0:1, :])
                nc.gpsimd.dma_start(out=dn[1:M, 1:3, :], in_=m2[0:M - 1, 1:3, :])
                nc.gpsimd.dma_start(out=sh[M - 1:M, :, :], in_=m2[M - 1:M, :, :])
                nc.sync.dma_start(out=sh[0:M - 1, 0:1, :], in_=m2[1:M, 0:1, :])
                nc.scalar.dma_start(out=sh[0:M - 1, 1:3, :], in_=m2[1:M, 1:3, :])

                # NMS (valid partitions [1, M-1), image cols [1, 639))
                p0 = pw.tile([128, NI, 638], F32, tag="p0")
                p1 = pw.tile([128, NI, 638], F32, tag="p1")
                p2 = pw.tile([128, NI, 638], F32, tag="p2")
                p3 = pw.tile([128, NI, 638], F32, tag="p3")
                nc.vector.tensor_tensor(out=p0[0:M, :, :], in0=m2[0:M, :, 0:638], in1=m2[0:M, :, 2:640], op=ALU.max)
                nc.vector.tensor_tensor(out=p1[0:M, :, :], in0=sh[0:M, :, 0:638], in1=dn[0:M, :, 2:640], op=ALU.max)
                nc.vector.tensor_tensor(out=p2[0:M, :, :], in0=sh[0:M, :, 1:639], in1=dn[0:M, :, 1:639], op=ALU.max)
                nc.vector.tensor_tensor(out=p3[0:M, :, :], in0=sh[0:M, :, 2:640], in1=dn[0:M, :, 0:638], op=ALU.max)

                pd = p3
                nc.vector.copy_predicated(out=pd[0:M, :, :], mask=sp[0:M, :, :].bitcast(mybir.dt.int16), data=p1[0:M, :, :])
                nc.vector.copy_predicated(out=pd[0:M, :, :], mask=b0m[0:M, :, :].bitcast(mybir.dt.int16), data=p0[0:M, :, :])
                nc.vector.copy_predicated(out=pd[0:M, :, :], mask=b2m[0:M, :, :].bitcast(mybir.dt.int16), data=p2[0:M, :, :])

                km = pw.tile([128, NI, 638], F32, tag="v1")
                nc.vector.tensor_tensor(out=km[0:M, :, :], in0=m2[0:M, :, 1:639], in1=pd[0:M, :, :], op=ALU.is_ge)
                nm = pw.tile([128, NI, 640], F32, tag="gx")
                nc.vector.tensor_tensor(out=nm[0:M, :, 1:639], in0=km[0:M, :, :], in1=m2[0:M, :, 1:639], op=ALU.mult)

                # borders + output
                nc.vector.memset(nm[0:M, :, 0:1], 0.0)
                nc.vector.memset(nm[0:M, :, 639:640], 0.0)
                if k == 0:
                    nc.gpsimd.memset(nm[0:1, :, :], 0.0)
                    plo, phi, rlo = 0, 120, 0
                elif k == 5:
                    plo, phi, rlo = 1, 40, 600
                else:
                    plo, phi, rlo = 1, 121, 120 * k
                for i in range(NI):
                    eng = (nc.sync, nc.scalar, nc.scalar)[i]
                    eng.dma_start(out=nms_d[i, rlo:rlo + (phi - plo), :], in_=nm[plo:phi, i, :])
                if k == 5:
                    for i in range(NI):
                        nc.sync.dma_start(out=nms_d[i, 639:640, :], in_=zrow[0:1, :])

            st = statp.tile([128, NI, 2], F32, tag="st")
            nc.vector.tensor_reduce(out=st[:, :, 0], in_=minsl[:, :, 0:6], axis=AXX, op=ALU.min)
            nc.vector.tensor_reduce(out=st[:, :, 1], in_=maxsl[:, :, 0:6], axis=AXX, op=ALU.max)
            for i in range(NI):
                nc.sync.dma_start(out=stats_d[i].rearrange("s p -> p s"), in_=st[:, i, :])
    nc.compile()
    return nc


def build_k2(tl2, th2):
    """tl2/th2: per-image (== per-channel) squared thresholds, device scale."""
    nc = bacc.Bacc()
    nms_d = nc.dram_tensor("nms2", [NI, H, H], F32, kind="ExternalInput")
    tri_d = nc.dram_tensor("tri", [128, 128], BF16, kind="ExternalInput")
    edges_d = nc.dram_tensor("edges", [NI, H, H], BF16, kind="ExternalOutput")
    th_in = [float(v) for v in th2]
    tl_in = [float(v) for v in tl2]
    assert len(th_in) == NI and len(tl_in) == NI

    with tile.TileContext(nc) as tc:
        with (
            tc.tile_pool(name="k2singles", bufs=1) as k2s,
            tc.tile_pool(name="work", bufs=4) as pw,
            tc.tile_pool(name="psum", bufs=2, space="PSUM") as psp,
        ):
            tri = k2s.tile([128, 128], BF16, tag="tri")
            nc.sync.dma_start(out=tri, in_=tri_d[:, :])
            for k, (lo, nl, poff, olo, nout) in enumerate(K2_STRIPS):
                PT = poff + nl + (1 if k == 5 else 0)   # wt partitions = rows [olo-1, olo+nout+1)
                wt = pw.tile([128, NI, 642], F32, tag="wt")
                nc.vector.memset(wt[0:PT, :, 0:1], 0.0)
                nc.vector.memset(wt[0:PT, :, 641:642], 0.0)
                if k == 0:
                    nc.vector.memset(wt[0:1, :, 1:641], 0.0)
                for i in range(NI):
                    eng = (nc.sync, nc.scalar, nc.gpsimd)[i]
                    if k == 5:
                        eng.dma_start(out=wt[11:12, i, 1:641], in_=nms_d[i, 639:640, :])
                    eng.dma_start(out=wt[poff:poff + nl, i, 1:641], in_=nms_d[i, lo:lo + nl, :])

                strong = pw.tile([128, NI, 642], BF16, tag="strong")
                wlo = pw.tile([128, NI, 642], BF16, tag="wlo")
                for i in range(NI):
                    nc.vector.tensor_scalar(out=strong[0:PT, i, :], in0=wt[0:PT, i, :], scalar1=th_in[i], scalar2=None, op0=ALU.is_ge)
                    nc.vector.tensor_scalar(out=wlo[0:PT, i, :], in0=wt[0:PT, i, :], scalar1=tl_in[i], scalar2=None, op0=ALU.is_ge)

                # horizontal dilation on DVE, vertical via PE tridiagonal band
                # (sum of binary neighbors > 0 == max, nonneg)
                d1 = pw.tile([128, NI, 640], BF16, tag="d1")
                nc.vector.tensor_tensor(out=d1[0:PT, :, :], in0=strong[0:PT, :, 0:640], in1=strong[0:PT, :, 2:642], op=ALU.max)
                h3 = pw.tile([128, NI, 640], BF16, tag="h3")
                nc.vector.tensor_tensor(out=h3[0:PT, :, :], in0=d1[0:PT, :, :], in1=strong[0:PT, :, 1:641], op=ALU.max)
                h3f = h3[0:PT, :, :].rearrange("p i c -> p (i c)")
                NF2 = NI * 640
                vd = psp.tile([128, NF2], F32, tag="vd")
                for c0_ in range(0, NF2, 512):
                    c1_ = min(c0_ + 512, NF2)
                    nc.tensor.matmul(out=vd[0:PT, c0_:c1_], lhsT=tri[0:PT, 0:PT], rhs=h3f[:, c0_:c1_], start=True, stop=True)

                q = pw.tile([128, NI, 640], BF16, tag="q")
                nc.vector.tensor_tensor(out=q[0:PT, :, :], in0=wlo[0:PT, :, 1:641], in1=strong[0:PT, :, 1:641], op=ALU.subtract)
                # vd >= 0 always, so Sign(vd) is exactly the binary dilation mask
                vdv = vd.rearrange("p (i c) -> p i c", i=NI)
                v3m = pw.tile([128, NI, 640], BF16, tag="v3m")
                nc.scalar.sign(out=v3m[0:PT, :, :], in_=vdv[0:PT, :, :])
                t2 = pw.tile([128, NI, 640], BF16, tag="t2")
                nc.vector.tensor_tensor(out=t2[0:PT, :, :], in0=v3m[0:PT, :, :], in1=q[0:PT, :, :], op=ALU.mult)
                ed = pw.tile([128, NI, 640], BF16, tag="ed")
                nc.vector.tensor_tensor(out=ed[0:PT, :, :], in0=strong[0:PT, :, 1:641], in1=t2[0:PT, :, :], op=ALU.add)

                for i in range(NI):
                    eng = (nc.sync, nc.scalar, nc.gpsimd)[i]
                    eng.dma_start(out=edges_d[i, olo:olo + nout, :], in_=ed[1:1 + nout, i, :])
    nc.compile()
    return nc


def kernel(x):
    x = np.asarray(x)
    assert x.shape == (NCORES, NI, H, H), x.shape
    xf = np.ascontiguousarray(x.astype(np.float32, copy=False))
    bands = _k1_band_inputs()

    nc1 = build_k1()
    in_maps1 = []
    for i in range(NCORES):
        m = {"x": np.ascontiguousarray(xf[i])}
        m.update(bands)
        in_maps1.append(m)
    r1 = run_bass_kernel_spmd(nc1, in_maps1, core_ids=list(range(NCORES)))
    nms2 = [np.asarray(r["nms2"]) for r in r1.results]
    stats = np.stack([np.asarray(r["stats"]) for r in r1.results])  # [8, 3, 2, 128]

    mn_dev = stats[:, :, 0, :].min(axis=(0, 2)).astype(np.float64)  # per channel
    mx_dev = stats[:, :, 1, :].max(axis=(0, 2)).astype(np.float64)
    mn = np.sqrt(mn_dev) * G1
    mx = np.sqrt(mx_dev) * G1
    tl = mn + 0.1 * (mx - mn + 1e-8)
    th = mn + 0.3 * (mx - mn + 1e-8)
    tl2 = np.float32((tl / G1) ** 2)
    th2 = np.float32((th / G1) ** 2)

    nc2 = build_k2(tl2, th2)
    tri = np.zeros((128, 128), np.float32)
    for d in (-1, 0, 1):
        i = np.arange(max(0, -d), min(128, 128 - d))
        tri[i + d, i] = 1.0          # lhsT[k, m] = 1 where |k - m| <= 1
    import ml_dtypes
    tri = tri.astype(ml_dtypes.bfloat16)
    in_maps2 = [{"nms2": np.ascontiguousarray(nms2[i]), "tri": tri} for i in range(NCORES)]
    r2 = run_bass_kernel_spmd(nc2, in_maps2, core_ids=list(range(NCORES)))
    edges = np.stack([np.asarray(r["edges"]).astype(np.float32) for r in r2.results])
    return edges
